# revision 7
# baseline (speedup 1.0000x reference)
"""BAP classifier (attention-pooling + linear head) on 8 TRN2 NeuronCores.

Pipeline (reference math):
    A    = sigmoid(einsum('bchw,mc->bmhw', x, Wa) + ba)     # attention maps
    bap  = einsum('bchw,bmhw->bmc', x, A) / (H*W)           # attn-weighted pool
    out  = bap.reshape(B, M*C) @ Wc.T + bc                  # linear head

Sharding:
  Phase 1 — data-parallel over batch (8 batches/core): each core computes
    raw feats rows [8, M*C] (un-normalized bap, transposed per batch on chip).
  Phase 2 — Wc column-parallel (8192 columns of the M*C dim per core): each
    core computes a partial [B, NCLS] logit; host sums partials, applies the
    1/(H*W) scale and bias.

Compute dtype is bf16 on the TensorEngine with fp32 PSUM accumulation.

Schedule notes (from NTFF traces):
  - each dma_start costs ~0.7us of issue time on its queue engine and a
    single queue sustains well under HBM rate, so transfers are spread
    over the sync/scalar (HWDGE) and gpsimd (SWDGE) queues;
  - the PE clock (HAM) throttles to 1.2 GHz after ~3.4us of idle, so the
    PE is kept warm with junk matmuls during the initial DMA ramp and the
    program order interleaves independent work into every dependency gap;
  - PSUM->SBUF drains split across Scalar and Vector so neither gates the
    bap accumulation banks.
"""
import sys

if "/opt/trn_rl_repo" not in sys.path:
    sys.path.insert(0, "/opt/trn_rl_repo")

import numpy as np

import concourse.bacc as bacc
import concourse.mybir as mybir
from concourse.tile import TileContext
from concourse.bass_utils import run_bass_kernel_spmd
from concourse.masks import make_identity

B, C, H, W = 64, 2048, 14, 14
HW = H * W                     # 196
M, NCLS = 32, 396
NCORES = 8
BPC = B // NCORES              # 8 batches per core
CT = C // 128                  # 16 c-chunks
KTOT = M * C                   # 65536
KPC = KTOT // NCORES           # 8192 Wc columns per core
KT = KPC // 128                # 64 k-tiles per core in phase 2

F32 = mybir.dt.float32
BF16 = mybir.dt.bfloat16

# Run options (test harness may flip these; defaults are what grading uses).
TRACE = False
TRACE_INFO = {}
TRACE_RES = {}

_cache = {}


def _nc():
    return bacc.Bacc(
        "TRN2", target_bir_lowering=False, debug=False, num_devices=NCORES
    )


def _build_phase1():
    """Per-core: x shard -> raw feats [BPC, M*C] (bf16).

    Inputs (host-permuted so every DMA descriptor is a contiguous >=4KB run):
      x01 [128, 2, CT, HW]     batches 0,1 (per-batch einsum1 for fast start)
      xp  [128, 3, CT, 2, HW]  batch pairs (2,3),(4,5),(6,7)
      xta [128, BPC, C]        x^T rows hw=0:128
      xtb [ 68, BPC, C]        x^T rows hw=128:196
      wat [128, CT, M]         Wa^T in the same permuted-c layout (c=p*CT+t)
      ba  [M, 1]
    """
    nc = _nc()
    x01 = nc.dram_tensor("x01", [128, 2, CT, HW], BF16, kind="ExternalInput")
    xp = nc.dram_tensor("xp", [128, 3, CT, 2, HW], BF16, kind="ExternalInput")
    xta = nc.dram_tensor("xta", [128, BPC, C], BF16, kind="ExternalInput")
    xtb = nc.dram_tensor("xtb", [68, BPC, C], BF16, kind="ExternalInput")
    wat = nc.dram_tensor("wat", [128, CT, M], BF16, kind="ExternalInput")
    ba = nc.dram_tensor("ba", [M, 1], F32, kind="ExternalInput")
    feats = nc.dram_tensor("feats", [BPC, M * C], BF16, kind="ExternalOutput")

    with TileContext(nc) as tc:
        with (
            tc.tile_pool(name="const", bufs=1) as const,
            tc.tile_pool(name="xpool", bufs=2) as xpool,
            tc.tile_pool(name="xtapool", bufs=8) as xtapool,
            tc.tile_pool(name="xtbpool", bufs=8) as xtbpool,
            tc.tile_pool(name="apool", bufs=3) as apool,
            tc.tile_pool(name="atpool", bufs=4) as atpool,
            tc.tile_pool(name="fpool", bufs=4) as fpool,
            tc.tile_pool(name="ps_att", bufs=2, space="PSUM") as ps_att,
            tc.tile_pool(name="ps_tr", bufs=1, space="PSUM") as ps_tr,
            tc.tile_pool(name="ps_bap", bufs=2, space="PSUM") as ps_bap,
        ):
            # PE warm-up source (memset on gpsimd before its loads)
            warm_sb = const.tile([128, 512], BF16)
            nc.gpsimd.memset(warm_sb, 0.0)
            ident = const.tile([M, M], BF16)
            make_identity(nc, ident)

            # weights + x on sync; x^T top rows on gpsimd; bottom on scalar
            wat_sb = const.tile([128, CT, M], BF16)
            nc.sync.dma_start(out=wat_sb, in_=wat.ap())
            ba_sb = const.tile([M, 1], F32)
            nc.sync.dma_start(out=ba_sb, in_=ba.ap())
            xs = []
            for b in range(2):
                x_b = xpool.tile(
                    [128, CT, HW], BF16, tag=f"x{b}", bufs=1, name=f"x{b}"
                )
                nc.sync.dma_start(out=x_b, in_=x01.ap()[:, b])
                xs.append(x_b)
            xps = []
            for i in range(3):
                x_p = xpool.tile(
                    [128, CT, 2, HW], BF16, tag="xp", bufs=3, name=f"xp{i}"
                )
                nc.sync.dma_start(out=x_p, in_=xp.ap()[:, i])
                xps.append(x_p)
            xtas, xtbs = [], []
            for b in range(BPC):
                xta_b = xtapool.tile([128, C], BF16, tag="xta", name=f"xta{b}")
                nc.gpsimd.dma_start(out=xta_b, in_=xta.ap()[:, b])
                xtas.append(xta_b)
            for b in range(BPC):
                xtb_b = xtbpool.tile([68, C], BF16, tag="xtb", name=f"xtb{b}")
                nc.scalar.dma_start(out=xtb_b, in_=xtb.ap()[:, b])
                xtbs.append(xtb_b)

            # keep the PE busy (and the HAM clock-gate open) during the
            # initial DMA ramp: junk matmuls on the memset tile.
            for i in range(16):
                w_ps = ps_att.tile([128, 512], F32, tag="att", name=f"warm{i}")
                nc.tensor.matmul(
                    w_ps, lhsT=warm_sb[:, 0:128], rhs=warm_sb, start=True,
                    stop=True, skip_group_check=True,
                )

            a_of = {}        # b -> (a_sb tile, hw-slicer)

            def emit_e1_single(b):
                att_ps = ps_att.tile([M, HW], F32, tag="att", name=f"att{b}")
                for ct in range(CT):
                    nc.tensor.matmul(
                        att_ps,
                        lhsT=wat_sb[:, ct, :],
                        rhs=xs[b][:, ct, :],
                        start=(ct == 0),
                        stop=(ct == CT - 1),
                    )
                a_sb = apool.tile([M, HW], BF16, tag="a_sb", name=f"a_sb{b}")
                nc.scalar.activation(
                    out=a_sb, in_=att_ps,
                    func=mybir.ActivationFunctionType.Sigmoid, bias=ba_sb,
                )
                a_of[b] = (a_sb, None)

            def emit_e1_pair(i):
                att_ps = ps_att.tile([M, 2, HW], F32, tag="att", name=f"attp{i}")
                for ct in range(CT):
                    nc.tensor.matmul(
                        att_ps,
                        lhsT=wat_sb[:, ct, :],
                        rhs=xps[i][:, ct, :, :],
                        start=(ct == 0),
                        stop=(ct == CT - 1),
                    )
                a_sb = apool.tile(
                    [M, 2, HW], BF16, tag="a_sb", name=f"a_sbp{i}"
                )
                nc.scalar.activation(
                    out=a_sb, in_=att_ps,
                    func=mybir.ActivationFunctionType.Sigmoid, bias=ba_sb,
                )
                a_of[2 + 2 * i] = (a_sb, 0)
                a_of[3 + 2 * i] = (a_sb, 1)

            ats = {}         # b -> (ata, atb)

            def emit_tr(b):
                a_sb, b2 = a_of[b]
                aa = a_sb[:, 0:128] if b2 is None else a_sb[:, b2, 0:128]
                ab = a_sb[:, 128:HW] if b2 is None else a_sb[:, b2, 128:HW]
                ata_ps = ps_tr.tile([128, M], BF16, tag="ata")
                nc.tensor.transpose(ata_ps, aa, ident)
                ata = atpool.tile([128, M], BF16, tag="ata_sb")
                nc.scalar.copy(out=ata, in_=ata_ps)
                atb_ps = ps_tr.tile([68, M], BF16, tag="atb")
                nc.tensor.transpose(atb_ps, ab, ident)
                atb = atpool.tile([68, M], BF16, tag="atb_sb")
                nc.vector.tensor_copy(out=atb, in_=atb_ps)
                ats[b] = (ata, atb)

            def emit_e2(b):
                ata, atb = ats[b]
                featsq = fpool.tile([M, C], BF16, tag="featsq", name=f"fq{b}")
                bap_ps = [
                    ps_bap.tile([M, 1024], F32, tag="bap", name=f"bap{b}_{h}")
                    for h in range(2)
                ]
                for h in range(2):
                    for nt in range(2):
                        c0 = 1024 * h + 512 * nt
                        nc.tensor.matmul(
                            bap_ps[h][:, 512 * nt : 512 * (nt + 1)],
                            lhsT=ata, rhs=xtas[b][:, c0 : c0 + 512],
                            start=True, stop=False,
                        )
                for h in range(2):
                    for nt in range(2):
                        c0 = 1024 * h + 512 * nt
                        nc.tensor.matmul(
                            bap_ps[h][:, 512 * nt : 512 * (nt + 1)],
                            lhsT=atb, rhs=xtbs[b][:, c0 : c0 + 512],
                            start=False, stop=True,
                        )
                # parallel drains: vector gets half 0, scalar gets half 1
                nc.vector.tensor_copy(out=featsq[:, 0:1024], in_=bap_ps[0])
                nc.scalar.copy(out=featsq[:, 1024:2048], in_=bap_ps[1])
                nc.scalar.dma_start(
                    out=feats.ap()[b : b + 1].rearrange(
                        "b (m c) -> (b m) c", m=M
                    ),
                    in_=featsq,
                )

            # PE program order: einsum1 runs ahead so transposes/einsum2
            # never leave the PE waiting on Scalar/Vector round trips.
            emit_e1_single(0)
            emit_e1_single(1)
            emit_tr(0)
            emit_e1_pair(0)
            emit_e2(0)
            emit_tr(1)
            emit_e2(1)
            emit_tr(2)
            emit_e1_pair(1)
            emit_e2(2)
            emit_tr(3)
            emit_e2(3)
            emit_tr(4)
            emit_e1_pair(2)
            emit_e2(4)
            emit_tr(5)
            emit_e2(5)
            emit_tr(6)
            emit_e2(6)
            emit_tr(7)
            emit_e2(7)
    nc.compile()
    return nc


def _build_phase2():
    """Per-core: featsT slice (partition-major, bf16) x WcT slice (bf16)
    -> partial [B, NCLS] (fp32)."""
    nc = _nc()
    ft = nc.dram_tensor("ft", [128, KT, B], BF16, kind="ExternalInput")
    wct = nc.dram_tensor("wct", [128, KT, NCLS], BF16, kind="ExternalInput")
    part = nc.dram_tensor("part", [B, NCLS], F32, kind="ExternalOutput")

    # graduated chunk sizes so the first matmul starts early, spread over
    # three queues; program-order consumption matches expected arrival.
    CHUNKS = [2, 2, 4, 4, 8, 8, 8, 8, 10, 10]
    QUEUES = ["scalar", "gpsimd", "scalar", "gpsimd", "scalar", "gpsimd",
              "sync", "scalar", "gpsimd", "sync"]
    NFT = 4

    with TileContext(nc) as tc:
        with (
            tc.tile_pool(name="const", bufs=1) as const,
            tc.tile_pool(name="fpool", bufs=NFT) as fpool,
            tc.tile_pool(name="wpool", bufs=len(CHUNKS)) as wpool,
            tc.tile_pool(name="opool", bufs=1) as opool,
            tc.tile_pool(name="ps_out", bufs=1, space="PSUM") as ps_out,
            tc.tile_pool(name="ps_warm", bufs=1, space="PSUM") as ps_warm,
        ):
            warm_sb = const.tile([128, 512], BF16)
            nc.gpsimd.memset(warm_sb, 0.0)

            ft_sb = []
            fstep = KT // NFT
            for i in range(NFT):
                t = fpool.tile([128, fstep, B], BF16, tag="ft", name=f"ft{i}")
                nc.sync.dma_start(
                    out=t, in_=ft.ap()[:, i * fstep : (i + 1) * fstep]
                )
                ft_sb.append(t)
            w_sbs = []
            k0 = 0
            for kc, ch in enumerate(CHUNKS):
                w_sb = wpool.tile(
                    [128, ch, NCLS], BF16, tag=f"w{kc}", bufs=1, name=f"w{kc}"
                )
                eng = getattr(nc, QUEUES[kc])
                eng.dma_start(out=w_sb, in_=wct.ap()[:, k0 : k0 + ch])
                w_sbs.append((k0, w_sb))
                k0 += ch

            for i in range(14):
                w_ps = ps_warm.tile([128, 512], F32, tag="warm", name=f"wm{i}")
                nc.tensor.matmul(
                    w_ps, lhsT=warm_sb[:, 0:128], rhs=warm_sb, start=True,
                    stop=True, skip_group_check=True,
                )

            out_ps = ps_out.tile([B, NCLS], F32)
            for k0, w_sb in w_sbs:
                for kl in range(w_sb.shape[1]):
                    kt = k0 + kl
                    nc.tensor.matmul(
                        out_ps,
                        lhsT=ft_sb[kt // fstep][:, kt % fstep, :],
                        rhs=w_sb[:, kl, :],
                        start=(kt == 0),
                        stop=(kt == KT - 1),
                    )
            out_sb = opool.tile([B, NCLS], F32)
            nc.vector.tensor_copy(out=out_sb, in_=out_ps)
            nc.sync.dma_start(out=part.ap(), in_=out_sb)
    nc.compile()
    return nc


def _install_ntff_hook():
    import types

    import trn_agent_boot.trn_boot as tb
    import concourse.bass_utils as bu

    hook = tb._ntff_profile_via_ctypes("/opt/axon/libaxon_pjrt.so")
    mod = types.ModuleType("antenv.axon_hooks")
    mod.get_axon_ntff_profile_hook = lambda: hook
    sys.modules["antenv.axon_hooks"] = mod
    bu.upload_artifacts = lambda tmpdir: "(skipped)"


def _run(nc, in_maps, label):
    core_ids = list(range(NCORES))
    if TRACE:
        _install_ntff_hook()
        res = run_bass_kernel_spmd(nc, in_maps, core_ids, trace=True)
        TRACE_INFO[label] = res.exec_time_ns
        TRACE_RES[label] = res
    else:
        res = run_bass_kernel_spmd(nc, in_maps, core_ids)
    return res.results


def kernel(x, Wa, ba, Wc, bc):
    import ml_dtypes

    bf16 = np.dtype(ml_dtypes.bfloat16)
    x3 = np.ascontiguousarray(x, dtype=np.float32).reshape(B, C, HW)
    xb = x3.astype(bf16)
    x4 = xb.reshape(B, 128, CT, HW)
    # x01[p, b, t, hw] / xp[p, i, t, b2, hw] with c = p*CT + t
    x01s = [
        np.ascontiguousarray(
            x4[i * BPC : i * BPC + 2].transpose(1, 0, 2, 3)
        )
        for i in range(NCORES)
    ]
    xps = [
        np.ascontiguousarray(
            x4[i * BPC + 2 : (i + 1) * BPC]
            .reshape(3, 2, 128, CT, HW)
            .transpose(2, 0, 3, 1, 4)
        )
        for i in range(NCORES)
    ]
    xt = xb.transpose(2, 0, 1)  # [HW, B, C]
    xta = np.ascontiguousarray(xt[0:128])
    xtb = np.ascontiguousarray(xt[128:196])
    # wat[p, t, m] = Wa[m, p*CT + t] — matches the kernel's permuted c layout
    wat = np.ascontiguousarray(Wa.T, dtype=np.float32).astype(bf16).reshape(
        128, CT, M
    )
    ba2 = np.ascontiguousarray(ba, dtype=np.float32).reshape(M, 1)
    wct = np.ascontiguousarray(Wc.T, dtype=np.float32).astype(bf16)  # [KTOT, NCLS]

    if "p1" not in _cache:
        _cache["p1"] = _build_phase1()
    if "p2" not in _cache:
        _cache["p2"] = _build_phase2()

    in1 = [
        {
            "x01": x01s[i],
            "xp": xps[i],
            "xta": xta[:, i * BPC : (i + 1) * BPC],
            "xtb": xtb[:, i * BPC : (i + 1) * BPC],
            "wat": wat,
            "ba": ba2,
        }
        for i in range(NCORES)
    ]
    res1 = _run(_cache["p1"], in1, "phase1")
    feats = np.concatenate([r["feats"] for r in res1], axis=0)  # [B, KTOT] bf16

    # ft[p, t, b] = feats[b, kslice + t*128 + p] (partition-major, bf16)
    featsT = np.ascontiguousarray(feats.T)  # [KTOT, B]
    in2 = [
        {
            "ft": np.ascontiguousarray(
                featsT[i * KPC : (i + 1) * KPC].reshape(KT, 128, B).transpose(
                    1, 0, 2
                )
            ),
            "wct": np.ascontiguousarray(
                wct[i * KPC : (i + 1) * KPC].reshape(KT, 128, NCLS).transpose(
                    1, 0, 2
                )
            ),
        }
        for i in range(NCORES)
    ]
    res2 = _run(_cache["p2"], in2, "phase2")
    parts = np.stack([r["part"] for r in res2], axis=0)  # [NCORES, B, NCLS]

    logits = parts.sum(axis=0) / float(HW) + np.asarray(bc, dtype=np.float32)
    return logits.astype(np.float32)


# revision 8
# speedup vs baseline: 1.1111x; 1.1111x over previous
"""BAP classifier (attention-pooling + linear head) on 8 TRN2 NeuronCores.

Pipeline (reference math):
    A    = sigmoid(einsum('bchw,mc->bmhw', x, Wa) + ba)     # attention maps
    bap  = einsum('bchw,bmhw->bmc', x, A) / (H*W)           # attn-weighted pool
    out  = bap.reshape(B, M*C) @ Wc.T + bc                  # linear head

Sharding:
  Phase 1 — data-parallel over batch (8 batches/core): each core computes
    raw feats rows [8, M*C] (un-normalized bap, transposed per batch on chip).
  Phase 2 — Wc column-parallel (8192 columns of the M*C dim per core): each
    core computes a partial [B, NCLS] logit; host sums partials, applies the
    1/(H*W) scale and bias.

Compute dtype is bf16 on the TensorEngine with fp32 PSUM accumulation.

Schedule notes (from NTFF traces):
  - each dma_start costs ~0.7us of issue time on its queue engine and a
    single queue sustains well under HBM rate, so transfers are spread
    over the sync/scalar (HWDGE) and gpsimd (SWDGE) queues;
  - the PE clock (HAM) throttles to 1.2 GHz after ~3.4us of idle, so the
    PE is kept warm with junk matmuls during the initial DMA ramp and the
    program order interleaves independent work into every dependency gap;
  - PSUM->SBUF drains split across Scalar and Vector so neither gates the
    bap accumulation banks.
"""
import sys

if "/opt/trn_rl_repo" not in sys.path:
    sys.path.insert(0, "/opt/trn_rl_repo")

import numpy as np

import concourse.bacc as bacc
import concourse.mybir as mybir
from concourse.tile import TileContext
from concourse.bass_utils import run_bass_kernel_spmd
from concourse.masks import make_identity

B, C, H, W = 64, 2048, 14, 14
HW = H * W                     # 196
M, NCLS = 32, 396
NCORES = 8
BPC = B // NCORES              # 8 batches per core
CT = C // 128                  # 16 c-chunks
KTOT = M * C                   # 65536
KPC = KTOT // NCORES           # 8192 Wc columns per core
KT = KPC // 128                # 64 k-tiles per core in phase 2

F32 = mybir.dt.float32
BF16 = mybir.dt.bfloat16

# Run options (test harness may flip these; defaults are what grading uses).
TRACE = False
TRACE_INFO = {}
TRACE_RES = {}

_cache = {}


def _nc():
    return bacc.Bacc(
        "TRN2", target_bir_lowering=False, debug=False, num_devices=NCORES
    )


def _build_phase1():
    """Per-core: x shard -> raw feats [BPC, M*C] (bf16).

    Inputs (host-permuted so every DMA descriptor is a contiguous >=4KB run):
      x01 [128, 2, CT, HW]     batches 0,1 (per-batch einsum1 for fast start)
      xp  [128, 3, CT, 2, HW]  batch pairs (2,3),(4,5),(6,7)
      xta [128, BPC, C]        x^T rows hw=0:128
      xtb [ 68, BPC, C]        x^T rows hw=128:196
      wat [128, CT, M]         Wa^T in the same permuted-c layout (c=p*CT+t)
      ba  [M, 1]
    """
    nc = _nc()
    x01 = nc.dram_tensor("x01", [128, 2, CT, HW], BF16, kind="ExternalInput")
    xp = nc.dram_tensor("xp", [128, 3, CT, 2, HW], BF16, kind="ExternalInput")
    xta = nc.dram_tensor("xta", [128, BPC, C], BF16, kind="ExternalInput")
    xtb = nc.dram_tensor("xtb", [68, BPC, C], BF16, kind="ExternalInput")
    # (x^T loaded in batch pairs: slices [:, 2p:2p+2] are contiguous)
    wat = nc.dram_tensor("wat", [128, CT, M], BF16, kind="ExternalInput")
    ba = nc.dram_tensor("ba", [M, 1], F32, kind="ExternalInput")
    feats = nc.dram_tensor("feats", [BPC, M * C], BF16, kind="ExternalOutput")

    with TileContext(nc) as tc:
        with (
            tc.tile_pool(name="const", bufs=1) as const,
            tc.tile_pool(name="xpool", bufs=2) as xpool,
            tc.tile_pool(name="xtapool", bufs=4) as xtapool,
            tc.tile_pool(name="xtbpool", bufs=4) as xtbpool,
            tc.tile_pool(name="apool", bufs=3) as apool,
            tc.tile_pool(name="atpool", bufs=4) as atpool,
            tc.tile_pool(name="fpool", bufs=4) as fpool,
            tc.tile_pool(name="ps_att", bufs=2, space="PSUM") as ps_att,
            tc.tile_pool(name="ps_tr", bufs=1, space="PSUM") as ps_tr,
            tc.tile_pool(name="ps_bap", bufs=2, space="PSUM") as ps_bap,
        ):
            # PE warm-up source (memset on gpsimd before its loads)
            warm_sb = const.tile([128, 512], BF16)
            nc.gpsimd.memset(warm_sb, 0.0)
            ident = const.tile([M, M], BF16)
            make_identity(nc, ident)

            # weights + x on sync; x^T top rows on gpsimd; bottom on scalar
            wat_sb = const.tile([128, CT, M], BF16)
            nc.sync.dma_start(out=wat_sb, in_=wat.ap())
            ba_sb = const.tile([M, 1], F32)
            nc.sync.dma_start(out=ba_sb, in_=ba.ap())
            xs = []
            for b in range(2):
                x_b = xpool.tile(
                    [128, CT, HW], BF16, tag=f"x{b}", bufs=1, name=f"x{b}"
                )
                nc.sync.dma_start(out=x_b, in_=x01.ap()[:, b])
                xs.append(x_b)
            xps = []
            for i in range(3):
                x_p = xpool.tile(
                    [128, CT, 2, HW], BF16, tag="xp", bufs=3, name=f"xp{i}"
                )
                nc.sync.dma_start(out=x_p, in_=xp.ap()[:, i])
                xps.append(x_p)
            xtas, xtbs = [], []
            for p in range(BPC // 2):
                xta_p = xtapool.tile(
                    [128, 2, C], BF16, tag="xta", name=f"xta{p}"
                )
                nc.gpsimd.dma_start(
                    out=xta_p, in_=xta.ap()[:, 2 * p : 2 * p + 2]
                )
                xtb_p = xtbpool.tile([68, 2, C], BF16, tag="xtb", name=f"xtb{p}")
                nc.gpsimd.dma_start(
                    out=xtb_p, in_=xtb.ap()[:, 2 * p : 2 * p + 2]
                )
                xtas.append(xta_p)
                xtbs.append(xtb_p)

            # keep the PE busy (and the HAM clock-gate open) during the
            # initial DMA ramp: one long junk accumulation chain (chained
            # MMs issue back-to-back with no inter-MM semaphores).
            w_ps = ps_att.tile([128, 512], F32, tag="att", name="warm")
            for i in range(16):
                nc.tensor.matmul(
                    w_ps, lhsT=warm_sb[:, 0:128], rhs=warm_sb,
                    start=(i == 0), stop=(i == 15),
                )

            a_of = {}        # b -> (a_sb tile, hw-slicer)

            def emit_e1_single(b):
                att_ps = ps_att.tile([M, HW], F32, tag="att", name=f"att{b}")
                for ct in range(CT):
                    nc.tensor.matmul(
                        att_ps,
                        lhsT=wat_sb[:, ct, :],
                        rhs=xs[b][:, ct, :],
                        start=(ct == 0),
                        stop=(ct == CT - 1),
                    )
                a_sb = apool.tile([M, HW], BF16, tag="a_sb", name=f"a_sb{b}")
                nc.scalar.activation(
                    out=a_sb, in_=att_ps,
                    func=mybir.ActivationFunctionType.Sigmoid, bias=ba_sb,
                )
                a_of[b] = (a_sb, None)

            def emit_e1_pair(i):
                att_ps = ps_att.tile([M, 2, HW], F32, tag="att", name=f"attp{i}")
                for ct in range(CT):
                    nc.tensor.matmul(
                        att_ps,
                        lhsT=wat_sb[:, ct, :],
                        rhs=xps[i][:, ct, :, :],
                        start=(ct == 0),
                        stop=(ct == CT - 1),
                    )
                a_sb = apool.tile(
                    [M, 2, HW], BF16, tag="a_sb", name=f"a_sbp{i}"
                )
                nc.scalar.activation(
                    out=a_sb, in_=att_ps,
                    func=mybir.ActivationFunctionType.Sigmoid, bias=ba_sb,
                )
                a_of[2 + 2 * i] = (a_sb, 0)
                a_of[3 + 2 * i] = (a_sb, 1)

            ats = {}         # b -> (ata, atb)

            def emit_tr(b):
                a_sb, b2 = a_of[b]
                aa = a_sb[:, 0:128] if b2 is None else a_sb[:, b2, 0:128]
                ab = a_sb[:, 128:HW] if b2 is None else a_sb[:, b2, 128:HW]
                ata_ps = ps_tr.tile([128, M], BF16, tag="ata")
                nc.tensor.transpose(ata_ps, aa, ident)
                ata = atpool.tile([128, M], BF16, tag="ata_sb")
                nc.scalar.copy(out=ata, in_=ata_ps)
                atb_ps = ps_tr.tile([68, M], BF16, tag="atb")
                nc.tensor.transpose(atb_ps, ab, ident)
                atb = atpool.tile([68, M], BF16, tag="atb_sb")
                nc.vector.tensor_copy(out=atb, in_=atb_ps)
                ats[b] = (ata, atb)

            def emit_e2(b):
                ata, atb = ats[b]
                xta_b = xtas[b // 2][:, b % 2]
                xtb_b = xtbs[b // 2][:, b % 2]
                featsq = fpool.tile([M, C], BF16, tag="featsq", name=f"fq{b}")
                bap_ps = [
                    ps_bap.tile([M, 1024], F32, tag="bap", name=f"bap{b}_{h}")
                    for h in range(2)
                ]
                for h in range(2):
                    for nt in range(2):
                        c0 = 1024 * h + 512 * nt
                        nc.tensor.matmul(
                            bap_ps[h][:, 512 * nt : 512 * (nt + 1)],
                            lhsT=ata, rhs=xta_b[:, c0 : c0 + 512],
                            start=True, stop=False,
                        )
                for h in range(2):
                    for nt in range(2):
                        c0 = 1024 * h + 512 * nt
                        nc.tensor.matmul(
                            bap_ps[h][:, 512 * nt : 512 * (nt + 1)],
                            lhsT=atb, rhs=xtb_b[:, c0 : c0 + 512],
                            start=False, stop=True,
                        )
                # parallel drains: vector gets half 0, scalar gets half 1
                nc.vector.tensor_copy(out=featsq[:, 0:1024], in_=bap_ps[0])
                nc.scalar.copy(out=featsq[:, 1024:2048], in_=bap_ps[1])
                nc.sync.dma_start(
                    out=feats.ap()[b : b + 1].rearrange(
                        "b (m c) -> (b m) c", m=M
                    ),
                    in_=featsq,
                )

            # PE program order: einsum1 runs ahead so transposes/einsum2
            # never leave the PE waiting on Scalar/Vector round trips.
            emit_e1_single(0)
            emit_e1_single(1)
            emit_tr(0)
            emit_e1_pair(0)
            emit_e2(0)
            emit_tr(1)
            emit_e2(1)
            emit_tr(2)
            emit_e1_pair(1)
            emit_e2(2)
            emit_tr(3)
            emit_e2(3)
            emit_tr(4)
            emit_e1_pair(2)
            emit_e2(4)
            emit_tr(5)
            emit_e2(5)
            emit_tr(6)
            emit_e2(6)
            emit_tr(7)
            emit_e2(7)
    nc.compile()
    return nc


def _build_phase2():
    """Per-core: featsT slice (partition-major, bf16) x WcT slice (bf16)
    -> partial [B, NCLS] (fp32)."""
    nc = _nc()
    ft = nc.dram_tensor("ft", [128, KT, B], BF16, kind="ExternalInput")
    wct = nc.dram_tensor("wct", [128, KT, NCLS], BF16, kind="ExternalInput")
    part = nc.dram_tensor("part", [B, NCLS], F32, kind="ExternalOutput")

    # graduated chunk sizes so the first matmul starts early, spread over
    # three queues; program-order consumption matches expected arrival.
    CHUNKS = [2, 2, 4, 4, 8, 8, 8, 8, 10, 10]
    QUEUES = ["scalar", "gpsimd", "scalar", "gpsimd", "scalar", "gpsimd",
              "scalar", "gpsimd", "scalar", "gpsimd"]
    NFT = 4

    with TileContext(nc) as tc:
        with (
            tc.tile_pool(name="const", bufs=1) as const,
            tc.tile_pool(name="fpool", bufs=NFT) as fpool,
            tc.tile_pool(name="wpool", bufs=len(CHUNKS)) as wpool,
            tc.tile_pool(name="opool", bufs=1) as opool,
            tc.tile_pool(name="ps_out", bufs=1, space="PSUM") as ps_out,
            tc.tile_pool(name="ps_warm", bufs=1, space="PSUM") as ps_warm,
        ):
            warm_sb = const.tile([128, 512], BF16)
            nc.gpsimd.memset(warm_sb, 0.0)

            ft_sb = []
            fstep = KT // NFT
            for i in range(NFT):
                t = fpool.tile([128, fstep, B], BF16, tag="ft", name=f"ft{i}")
                nc.sync.dma_start(
                    out=t, in_=ft.ap()[:, i * fstep : (i + 1) * fstep]
                )
                ft_sb.append(t)
            w_sbs = []
            k0 = 0
            for kc, ch in enumerate(CHUNKS):
                w_sb = wpool.tile(
                    [128, ch, NCLS], BF16, tag=f"w{kc}", bufs=1, name=f"w{kc}"
                )
                eng = getattr(nc, QUEUES[kc])
                eng.dma_start(out=w_sb, in_=wct.ap()[:, k0 : k0 + ch])
                w_sbs.append((k0, w_sb))
                k0 += ch

            w_ps = ps_warm.tile([128, 512], F32, tag="warm", name="wm")
            for i in range(14):
                nc.tensor.matmul(
                    w_ps, lhsT=warm_sb[:, 0:128], rhs=warm_sb,
                    start=(i == 0), stop=(i == 13),
                )

            out_ps = ps_out.tile([B, NCLS], F32)
            for k0, w_sb in w_sbs:
                for kl in range(w_sb.shape[1]):
                    kt = k0 + kl
                    nc.tensor.matmul(
                        out_ps,
                        lhsT=ft_sb[kt // fstep][:, kt % fstep, :],
                        rhs=w_sb[:, kl, :],
                        start=(kt == 0),
                        stop=(kt == KT - 1),
                    )
            out_sb = opool.tile([B, NCLS], F32)
            nc.vector.tensor_copy(out=out_sb, in_=out_ps)
            nc.sync.dma_start(out=part.ap(), in_=out_sb)
    nc.compile()
    return nc


def _install_ntff_hook():
    import types

    import trn_agent_boot.trn_boot as tb
    import concourse.bass_utils as bu

    hook = tb._ntff_profile_via_ctypes("/opt/axon/libaxon_pjrt.so")
    mod = types.ModuleType("antenv.axon_hooks")
    mod.get_axon_ntff_profile_hook = lambda: hook
    sys.modules["antenv.axon_hooks"] = mod
    bu.upload_artifacts = lambda tmpdir: "(skipped)"


def _run(nc, in_maps, label):
    core_ids = list(range(NCORES))
    if TRACE:
        _install_ntff_hook()
        res = run_bass_kernel_spmd(nc, in_maps, core_ids, trace=True)
        TRACE_INFO[label] = res.exec_time_ns
        TRACE_RES[label] = res
    else:
        res = run_bass_kernel_spmd(nc, in_maps, core_ids)
    return res.results


def kernel(x, Wa, ba, Wc, bc):
    import ml_dtypes

    bf16 = np.dtype(ml_dtypes.bfloat16)
    x3 = np.ascontiguousarray(x, dtype=np.float32).reshape(B, C, HW)
    xb = x3.astype(bf16)
    x4 = xb.reshape(B, 128, CT, HW)
    # x01[p, b, t, hw] / xp[p, i, t, b2, hw] with c = p*CT + t
    x01s = [
        np.ascontiguousarray(
            x4[i * BPC : i * BPC + 2].transpose(1, 0, 2, 3)
        )
        for i in range(NCORES)
    ]
    xps = [
        np.ascontiguousarray(
            x4[i * BPC + 2 : (i + 1) * BPC]
            .reshape(3, 2, 128, CT, HW)
            .transpose(2, 0, 3, 1, 4)
        )
        for i in range(NCORES)
    ]
    xt = xb.transpose(2, 0, 1)  # [HW, B, C]
    xta = np.ascontiguousarray(xt[0:128])
    xtb = np.ascontiguousarray(xt[128:196])
    # wat[p, t, m] = Wa[m, p*CT + t] — matches the kernel's permuted c layout
    wat = np.ascontiguousarray(Wa.T, dtype=np.float32).astype(bf16).reshape(
        128, CT, M
    )
    ba2 = np.ascontiguousarray(ba, dtype=np.float32).reshape(M, 1)
    wct = np.ascontiguousarray(Wc.T, dtype=np.float32).astype(bf16)  # [KTOT, NCLS]

    if "p1" not in _cache:
        _cache["p1"] = _build_phase1()
    if "p2" not in _cache:
        _cache["p2"] = _build_phase2()

    in1 = [
        {
            "x01": x01s[i],
            "xp": xps[i],
            "xta": xta[:, i * BPC : (i + 1) * BPC],
            "xtb": xtb[:, i * BPC : (i + 1) * BPC],
            "wat": wat,
            "ba": ba2,
        }
        for i in range(NCORES)
    ]
    res1 = _run(_cache["p1"], in1, "phase1")
    feats = np.concatenate([r["feats"] for r in res1], axis=0)  # [B, KTOT] bf16

    # ft[p, t, b] = feats[b, kslice + t*128 + p] (partition-major, bf16)
    featsT = np.ascontiguousarray(feats.T)  # [KTOT, B]
    in2 = [
        {
            "ft": np.ascontiguousarray(
                featsT[i * KPC : (i + 1) * KPC].reshape(KT, 128, B).transpose(
                    1, 0, 2
                )
            ),
            "wct": np.ascontiguousarray(
                wct[i * KPC : (i + 1) * KPC].reshape(KT, 128, NCLS).transpose(
                    1, 0, 2
                )
            ),
        }
        for i in range(NCORES)
    ]
    res2 = _run(_cache["p2"], in2, "phase2")
    parts = np.stack([r["part"] for r in res2], axis=0)  # [NCORES, B, NCLS]

    logits = parts.sum(axis=0) / float(HW) + np.asarray(bc, dtype=np.float32)
    return logits.astype(np.float32)


# revision 9
# speedup vs baseline: 1.1973x; 1.0775x over previous
"""BAP classifier (attention-pooling + linear head) on 8 TRN2 NeuronCores.

Pipeline (reference math):
    A    = sigmoid(einsum('bchw,mc->bmhw', x, Wa) + ba)     # attention maps
    bap  = einsum('bchw,bmhw->bmc', x, A) / (H*W)           # attn-weighted pool
    out  = bap.reshape(B, M*C) @ Wc.T + bc                  # linear head

Sharding:
  Phase 1 — data-parallel over batch (8 batches/core): each core computes
    raw feats rows [8, M*C] (un-normalized bap, transposed per batch on chip).
  Phase 2 — Wc column-parallel (8192 columns of the M*C dim per core): each
    core computes a partial [B, NCLS] logit; host sums partials, applies the
    1/(H*W) scale and bias.

Compute dtype is bf16 on the TensorEngine with fp32 PSUM accumulation.

Schedule notes (from NTFF traces):
  - each dma_start costs ~0.7us of issue time on its queue engine and a
    single queue sustains well under HBM rate, so transfers are spread
    over the sync/scalar (HWDGE) and gpsimd (SWDGE) queues;
  - the PE clock (HAM) throttles to 1.2 GHz after ~3.4us of idle, so the
    PE is kept warm with junk matmuls during the initial DMA ramp and the
    program order interleaves independent work into every dependency gap;
  - PSUM->SBUF drains split across Scalar and Vector so neither gates the
    bap accumulation banks.
"""
import sys

if "/opt/trn_rl_repo" not in sys.path:
    sys.path.insert(0, "/opt/trn_rl_repo")

import numpy as np

import concourse.bacc as bacc
import concourse.mybir as mybir
from concourse.tile import TileContext
from concourse.bass_utils import run_bass_kernel_spmd
from concourse.masks import make_identity

B, C, H, W = 64, 2048, 14, 14
HW = H * W                     # 196
M, NCLS = 32, 396
NCORES = 8
BPC = B // NCORES              # 8 batches per core
CT = C // 128                  # 16 c-chunks
KTOT = M * C                   # 65536
KPC = KTOT // NCORES           # 8192 Wc columns per core
KT = KPC // 128                # 64 k-tiles per core in phase 2

F32 = mybir.dt.float32
BF16 = mybir.dt.bfloat16

# Run options (test harness may flip these; defaults are what grading uses).
TRACE = False
TRACE_INFO = {}
TRACE_RES = {}

_cache = {}


def _nc():
    return bacc.Bacc(
        "TRN2", target_bir_lowering=False, debug=False, num_devices=NCORES
    )


def _build_phase1():
    """Per-core: x shard -> raw feats [BPC, M*C] (bf16).

    Inputs (host-permuted so every DMA descriptor is a contiguous >=4KB run):
      x01 [128, 2, CT, HW]     batches 0,1 (per-batch einsum1 for fast start)
      xp  [128, 3, CT, 2, HW]  batch pairs (2,3),(4,5),(6,7)
      xta [128, BPC, C]        x^T rows hw=0:128
      xtb [ 68, BPC, C]        x^T rows hw=128:196
      wat [128, CT, M]         Wa^T in the same permuted-c layout (c=p*CT+t)
      ba  [M, 1]
    """
    nc = _nc()
    x01 = nc.dram_tensor("x01", [128, 2, CT, HW], BF16, kind="ExternalInput")
    xp = nc.dram_tensor("xp", [128, 3, CT, 2, HW], BF16, kind="ExternalInput")
    xta = nc.dram_tensor("xta", [128, BPC, C], BF16, kind="ExternalInput")
    xtb = nc.dram_tensor("xtb", [68, BPC, C], BF16, kind="ExternalInput")
    # (x^T loaded in batch pairs: slices [:, 2p:2p+2] are contiguous)
    wat = nc.dram_tensor("wat", [128, CT, M], BF16, kind="ExternalInput")
    ba = nc.dram_tensor("ba", [M, 1], F32, kind="ExternalInput")
    feats = nc.dram_tensor("feats", [BPC, M * C], BF16, kind="ExternalOutput")

    with TileContext(nc) as tc:
        with (
            tc.tile_pool(name="const", bufs=1) as const,
            tc.tile_pool(name="xpool", bufs=2) as xpool,
            tc.tile_pool(name="xtapool", bufs=4) as xtapool,
            tc.tile_pool(name="xtbpool", bufs=4) as xtbpool,
            tc.tile_pool(name="apool", bufs=3) as apool,
            tc.tile_pool(name="atpool", bufs=4) as atpool,
            tc.tile_pool(name="fpool", bufs=4) as fpool,
            tc.tile_pool(name="ps_att", bufs=2, space="PSUM") as ps_att,
            tc.tile_pool(name="ps_tr", bufs=1, space="PSUM") as ps_tr,
            tc.tile_pool(name="ps_bap", bufs=2, space="PSUM") as ps_bap,
        ):
            # PE warm-up source (memset on gpsimd before its loads)
            warm_sb = const.tile([128, 512], BF16)
            nc.gpsimd.memset(warm_sb, 0.0)
            ident = const.tile([M, M], BF16)
            make_identity(nc, ident)

            # ALL loads on the sync queue, issued in exact consumption
            # order: one HWDGE queue sustains the full ~370 GB/s and FIFO
            # order guarantees arrival matches the compute schedule.
            wat_sb = const.tile([128, CT, M], BF16)
            nc.sync.dma_start(out=wat_sb, in_=wat.ap())
            ba_sb = const.tile([M, 1], F32)
            nc.sync.dma_start(out=ba_sb, in_=ba.ap())
            xs = []
            for b in range(2):
                x_b = xpool.tile(
                    [128, CT, HW], BF16, tag=f"x{b}", bufs=1, name=f"x{b}"
                )
                nc.sync.dma_start(out=x_b, in_=x01.ap()[:, b])
                xs.append(x_b)
            xps, xtas, xtbs = [], [], []
            for p in range(BPC // 2):
                xta_p = xtapool.tile(
                    [128, 2, C], BF16, tag="xta", name=f"xta{p}"
                )
                nc.sync.dma_start(
                    out=xta_p, in_=xta.ap()[:, 2 * p : 2 * p + 2]
                )
                xtb_p = xtbpool.tile([68, 2, C], BF16, tag="xtb", name=f"xtb{p}")
                nc.sync.dma_start(
                    out=xtb_p, in_=xtb.ap()[:, 2 * p : 2 * p + 2]
                )
                xtas.append(xta_p)
                xtbs.append(xtb_p)
                if p < 3:
                    x_p = xpool.tile(
                        [128, CT, 2, HW], BF16, tag="xp", bufs=3, name=f"xp{p}"
                    )
                    nc.sync.dma_start(out=x_p, in_=xp.ap()[:, p])
                    xps.append(x_p)

            # keep the PE busy (and the HAM clock-gate open) during the
            # initial DMA ramp: one long junk accumulation chain (chained
            # MMs issue back-to-back with no inter-MM semaphores).
            w_ps = ps_att.tile([128, 512], F32, tag="att", name="warm")
            for i in range(16):
                nc.tensor.matmul(
                    w_ps, lhsT=warm_sb[:, 0:128], rhs=warm_sb,
                    start=(i == 0), stop=(i == 15),
                )

            a_of = {}        # b -> (a_sb tile, hw-slicer)

            def emit_e1_single(b):
                att_ps = ps_att.tile([M, HW], F32, tag="att", name=f"att{b}")
                for ct in range(CT):
                    nc.tensor.matmul(
                        att_ps,
                        lhsT=wat_sb[:, ct, :],
                        rhs=xs[b][:, ct, :],
                        start=(ct == 0),
                        stop=(ct == CT - 1),
                    )
                a_sb = apool.tile([M, HW], BF16, tag="a_sb", name=f"a_sb{b}")
                nc.scalar.activation(
                    out=a_sb, in_=att_ps,
                    func=mybir.ActivationFunctionType.Sigmoid, bias=ba_sb,
                )
                a_of[b] = (a_sb, None)

            def emit_e1_pair(i):
                att_ps = ps_att.tile([M, 2, HW], F32, tag="att", name=f"attp{i}")
                for ct in range(CT):
                    nc.tensor.matmul(
                        att_ps,
                        lhsT=wat_sb[:, ct, :],
                        rhs=xps[i][:, ct, :, :],
                        start=(ct == 0),
                        stop=(ct == CT - 1),
                    )
                a_sb = apool.tile(
                    [M, 2, HW], BF16, tag="a_sb", name=f"a_sbp{i}"
                )
                nc.scalar.activation(
                    out=a_sb, in_=att_ps,
                    func=mybir.ActivationFunctionType.Sigmoid, bias=ba_sb,
                )
                a_of[2 + 2 * i] = (a_sb, 0)
                a_of[3 + 2 * i] = (a_sb, 1)

            ats = {}         # b -> (ata, atb)

            def emit_tr(b):
                a_sb, b2 = a_of[b]
                aa = a_sb[:, 0:128] if b2 is None else a_sb[:, b2, 0:128]
                ab = a_sb[:, 128:HW] if b2 is None else a_sb[:, b2, 128:HW]
                ata_ps = ps_tr.tile([128, M], BF16, tag="ata")
                nc.tensor.transpose(ata_ps, aa, ident)
                ata = atpool.tile([128, M], BF16, tag="ata_sb")
                nc.scalar.copy(out=ata, in_=ata_ps)
                atb_ps = ps_tr.tile([68, M], BF16, tag="atb")
                nc.tensor.transpose(atb_ps, ab, ident)
                atb = atpool.tile([68, M], BF16, tag="atb_sb")
                nc.vector.tensor_copy(out=atb, in_=atb_ps)
                ats[b] = (ata, atb)

            def emit_e2(b):
                ata, atb = ats[b]
                xta_b = xtas[b // 2][:, b % 2]
                xtb_b = xtbs[b // 2][:, b % 2]
                featsq = fpool.tile([M, C], BF16, tag="featsq", name=f"fq{b}")
                bap_ps = [
                    ps_bap.tile([M, 1024], F32, tag="bap", name=f"bap{b}_{h}")
                    for h in range(2)
                ]
                for h in range(2):
                    for nt in range(2):
                        c0 = 1024 * h + 512 * nt
                        nc.tensor.matmul(
                            bap_ps[h][:, 512 * nt : 512 * (nt + 1)],
                            lhsT=ata, rhs=xta_b[:, c0 : c0 + 512],
                            start=True, stop=False,
                        )
                for h in range(2):
                    for nt in range(2):
                        c0 = 1024 * h + 512 * nt
                        nc.tensor.matmul(
                            bap_ps[h][:, 512 * nt : 512 * (nt + 1)],
                            lhsT=atb, rhs=xtb_b[:, c0 : c0 + 512],
                            start=False, stop=True,
                        )
                # parallel drains: vector gets half 0, scalar gets half 1
                nc.vector.tensor_copy(out=featsq[:, 0:1024], in_=bap_ps[0])
                nc.scalar.copy(out=featsq[:, 1024:2048], in_=bap_ps[1])
                nc.scalar.dma_start(
                    out=feats.ap()[b : b + 1].rearrange(
                        "b (m c) -> (b m) c", m=M
                    ),
                    in_=featsq,
                )

            # PE program order: einsum1 runs ahead so transposes/einsum2
            # never leave the PE waiting on Scalar/Vector round trips.
            emit_e1_single(0)
            emit_e1_single(1)
            emit_tr(0)
            emit_e1_pair(0)
            emit_e2(0)
            emit_tr(1)
            emit_e2(1)
            emit_tr(2)
            emit_e1_pair(1)
            emit_e2(2)
            emit_tr(3)
            emit_e2(3)
            emit_tr(4)
            emit_e1_pair(2)
            emit_e2(4)
            emit_tr(5)
            emit_e2(5)
            emit_tr(6)
            emit_e2(6)
            emit_tr(7)
            emit_e2(7)
    nc.compile()
    return nc


def _build_phase2():
    """Per-core: featsT slice (partition-major, bf16) x WcT slice (bf16)
    -> partial [B, NCLS] (fp32)."""
    nc = _nc()
    ft = nc.dram_tensor("ft", [128, KT, B], BF16, kind="ExternalInput")
    wct = nc.dram_tensor("wct", [128, KT, NCLS], BF16, kind="ExternalInput")
    part = nc.dram_tensor("part", [B, NCLS], F32, kind="ExternalOutput")

    # graduated chunk sizes so the first matmul starts early; everything
    # on the sync queue in exact consumption order (ft pieces interleaved
    # right before the chunks that need them; no chunk crosses an ft
    # boundary).  Output store on scalar so it can't block the load ring.
    CHUNKS = [2, 2, 4, 4, 4, 8, 8, 8, 8, 8, 6, 2]
    FSTEP = 16

    with TileContext(nc) as tc:
        with (
            tc.tile_pool(name="const", bufs=1) as const,
            tc.tile_pool(name="fpool", bufs=4) as fpool,
            tc.tile_pool(name="wpool", bufs=len(CHUNKS)) as wpool,
            tc.tile_pool(name="opool", bufs=1) as opool,
            tc.tile_pool(name="ps_out", bufs=1, space="PSUM") as ps_out,
            tc.tile_pool(name="ps_warm", bufs=1, space="PSUM") as ps_warm,
        ):
            warm_sb = const.tile([128, 512], BF16)
            nc.gpsimd.memset(warm_sb, 0.0)

            ft_sb = []

            def load_ft(i):
                t = fpool.tile([128, FSTEP, B], BF16, tag="ft", name=f"ft{i}")
                nc.sync.dma_start(
                    out=t, in_=ft.ap()[:, i * FSTEP : (i + 1) * FSTEP]
                )
                ft_sb.append(t)

            load_ft(0)
            w_sbs = []
            k0 = 0
            for kc, ch in enumerate(CHUNKS):
                if k0 + ch > FSTEP * len(ft_sb):
                    load_ft(len(ft_sb))
                w_sb = wpool.tile(
                    [128, ch, NCLS], BF16, tag=f"w{kc}", bufs=1, name=f"w{kc}"
                )
                nc.sync.dma_start(out=w_sb, in_=wct.ap()[:, k0 : k0 + ch])
                w_sbs.append((k0, w_sb))
                k0 += ch

            w_ps = ps_warm.tile([128, 512], F32, tag="warm", name="wm")
            for i in range(14):
                nc.tensor.matmul(
                    w_ps, lhsT=warm_sb[:, 0:128], rhs=warm_sb,
                    start=(i == 0), stop=(i == 13),
                )

            out_ps = ps_out.tile([B, NCLS], F32)
            for k0, w_sb in w_sbs:
                for kl in range(w_sb.shape[1]):
                    kt = k0 + kl
                    nc.tensor.matmul(
                        out_ps,
                        lhsT=ft_sb[kt // FSTEP][:, kt % FSTEP, :],
                        rhs=w_sb[:, kl, :],
                        start=(kt == 0),
                        stop=(kt == KT - 1),
                    )
            out_sb = opool.tile([B, NCLS], F32)
            nc.vector.tensor_copy(out=out_sb, in_=out_ps)
            nc.scalar.dma_start(out=part.ap(), in_=out_sb)
    nc.compile()
    return nc


def _install_ntff_hook():
    import types

    import trn_agent_boot.trn_boot as tb
    import concourse.bass_utils as bu

    hook = tb._ntff_profile_via_ctypes("/opt/axon/libaxon_pjrt.so")
    mod = types.ModuleType("antenv.axon_hooks")
    mod.get_axon_ntff_profile_hook = lambda: hook
    sys.modules["antenv.axon_hooks"] = mod
    bu.upload_artifacts = lambda tmpdir: "(skipped)"


def _run(nc, in_maps, label):
    core_ids = list(range(NCORES))
    if TRACE:
        _install_ntff_hook()
        res = run_bass_kernel_spmd(nc, in_maps, core_ids, trace=True)
        TRACE_INFO[label] = res.exec_time_ns
        TRACE_RES[label] = res
    else:
        res = run_bass_kernel_spmd(nc, in_maps, core_ids)
    return res.results


def kernel(x, Wa, ba, Wc, bc):
    import ml_dtypes

    bf16 = np.dtype(ml_dtypes.bfloat16)
    x3 = np.ascontiguousarray(x, dtype=np.float32).reshape(B, C, HW)
    xb = x3.astype(bf16)
    x4 = xb.reshape(B, 128, CT, HW)
    # x01[p, b, t, hw] / xp[p, i, t, b2, hw] with c = p*CT + t
    x01s = [
        np.ascontiguousarray(
            x4[i * BPC : i * BPC + 2].transpose(1, 0, 2, 3)
        )
        for i in range(NCORES)
    ]
    xps = [
        np.ascontiguousarray(
            x4[i * BPC + 2 : (i + 1) * BPC]
            .reshape(3, 2, 128, CT, HW)
            .transpose(2, 0, 3, 1, 4)
        )
        for i in range(NCORES)
    ]
    xt = xb.transpose(2, 0, 1)  # [HW, B, C]
    xta = np.ascontiguousarray(xt[0:128])
    xtb = np.ascontiguousarray(xt[128:196])
    # wat[p, t, m] = Wa[m, p*CT + t] — matches the kernel's permuted c layout
    wat = np.ascontiguousarray(Wa.T, dtype=np.float32).astype(bf16).reshape(
        128, CT, M
    )
    ba2 = np.ascontiguousarray(ba, dtype=np.float32).reshape(M, 1)
    wct = np.ascontiguousarray(Wc.T, dtype=np.float32).astype(bf16)  # [KTOT, NCLS]

    if "p1" not in _cache:
        _cache["p1"] = _build_phase1()
    if "p2" not in _cache:
        _cache["p2"] = _build_phase2()

    in1 = [
        {
            "x01": x01s[i],
            "xp": xps[i],
            "xta": xta[:, i * BPC : (i + 1) * BPC],
            "xtb": xtb[:, i * BPC : (i + 1) * BPC],
            "wat": wat,
            "ba": ba2,
        }
        for i in range(NCORES)
    ]
    res1 = _run(_cache["p1"], in1, "phase1")
    feats = np.concatenate([r["feats"] for r in res1], axis=0)  # [B, KTOT] bf16

    # ft[p, t, b] = feats[b, kslice + t*128 + p] (partition-major, bf16)
    featsT = np.ascontiguousarray(feats.T)  # [KTOT, B]
    in2 = [
        {
            "ft": np.ascontiguousarray(
                featsT[i * KPC : (i + 1) * KPC].reshape(KT, 128, B).transpose(
                    1, 0, 2
                )
            ),
            "wct": np.ascontiguousarray(
                wct[i * KPC : (i + 1) * KPC].reshape(KT, 128, NCLS).transpose(
                    1, 0, 2
                )
            ),
        }
        for i in range(NCORES)
    ]
    res2 = _run(_cache["p2"], in2, "phase2")
    parts = np.stack([r["part"] for r in res2], axis=0)  # [NCORES, B, NCLS]

    logits = parts.sum(axis=0) / float(HW) + np.asarray(bc, dtype=np.float32)
    return logits.astype(np.float32)
